# revision 1
# baseline (speedup 1.0000x reference)
"""BiLSTM (2-layer, bidirectional) Trainium2 kernel.

Strategy (multi-launch, 8 NeuronCores):
  L1: input projection pre0 = x @ W_ih[0,d].T for both dirs — 8 cores,
      each core does one (direction, time-quarter) slice as a dense GEMM.
  L2: layer-0 recurrences — core 0 forward, core 1 backward (backward fed
      time-reversed data, so both run the identical program).
  L3: projection pre1 = concat(h0f, h0b) @ W_ih[1,d].T — same GEMM NEFF.
  L4: layer-1 recurrences — same recurrence NEFF as L2.
Host does bias-add, gate permutation packing, time reversal, concat.

Numerics: matmuls in bf16 (projections) and float32r (recurrent path),
fp32 PSUM accumulation, fp32 cell state. Sigmoid computed as tanh-only
(C=2c / H=2h scaling trick) to avoid ACT table swaps.
"""

import sys

if "/opt/trn_rl_repo" not in sys.path:
    sys.path.insert(0, "/opt/trn_rl_repo")

from contextlib import ExitStack

import numpy as np
import ml_dtypes

import concourse.bass as bass
import concourse.mybir as mybir
import concourse.tile as tile
from concourse import bacc
from concourse.bass_utils import run_bass_kernel_spmd

F32 = mybir.dt.float32
F32R = mybir.dt.float32r
BF16 = mybir.dt.bfloat16
TANH = mybir.ActivationFunctionType.Tanh
MULT = mybir.AluOpType.mult
ADD = mybir.AluOpType.add

SEQ, BATCH, IN = 512, 64, 1024
H, G = 512, 2048
TQ = SEQ // 4  # 128 timesteps per GEMM core
TOK = TQ * BATCH  # 8192 tokens per GEMM core
REC_WIN = 8

_cache = {}


# ----------------------------------------------------------------- builders

def build_gemm():
    """Per core: out[8192, 2048] bf16 = xT.T @ wT   (K=1024)."""
    nc = bacc.Bacc("TRN2", target_bir_lowering=False, debug=False, num_devices=8)
    xT_d = nc.dram_tensor("xT", [IN, TOK], BF16, kind="ExternalInput").ap()
    wT_d = nc.dram_tensor("wT", [128, 8, G], BF16, kind="ExternalInput").ap()
    out_d = nc.dram_tensor("out", [TOK, G], BF16, kind="ExternalOutput").ap()
    with tile.TileContext(nc) as tc, ExitStack() as ctx:
        sb = ctx.enter_context(tc.tile_pool(name="sb", bufs=1))
        xb = ctx.enter_context(tc.tile_pool(name="xb", bufs=3))
        ob = ctx.enter_context(tc.tile_pool(name="ob", bufs=3))
        ps = ctx.enter_context(tc.tile_pool(name="ps", bufs=2, space="PSUM"))
        wT = sb.tile([128, 8, G], BF16)
        nc.sync.dma_start(out=wT, in_=wT_d)
        for m in range(TOK // 128):
            xt = xb.tile([128, 8, 128], BF16, tag="xt")
            nc.sync.dma_start(
                out=xt,
                in_=xT_d[:, 128 * m : 128 * m + 128].rearrange(
                    "(k p) t -> p k t", p=128
                ),
            )
            ot = ob.tile([128, 4, 512], BF16, tag="ot")
            for g in range(4):
                psum = ps.tile([128, 512], F32, tag="ps")
                for k in range(8):
                    nc.tensor.matmul(
                        psum, xt[:, k, :], wT[:, k, 512 * g : 512 * g + 512],
                        start=(k == 0), stop=(k == 7),
                    )
                nc.vector.tensor_copy(ot[:, g, :], psum)
            nc.sync.dma_start(out=out_d[128 * m : 128 * m + 128, :],
                              in_=ot.rearrange("p a b -> p (a b)"))
    nc.compile()
    return nc


def build_rec(T=128, win=REC_WIN):
    """Recurrence: see module docstring of the development history; runs on 2 cores."""
    nc = bacc.Bacc("TRN2", target_bir_lowering=False, debug=False, num_devices=2)
    pre_d = nc.dram_tensor("pre", [T, 64, 4, 512], BF16, kind="ExternalInput").ap()
    whT_d = nc.dram_tensor("whT", [128, 4, G], F32R, kind="ExternalInput").ap()
    idab_d = nc.dram_tensor("idab", [64, 64], BF16, kind="ExternalInput").ap()
    idf_d = nc.dram_tensor("idf", [64, 64], F32, kind="ExternalInput").ap()
    hT0_d = nc.dram_tensor("hT0", [128, 4, 64], F32R, kind="ExternalInput").ap()
    C0_d = nc.dram_tensor("C0", [64, 4, 128], F32, kind="ExternalInput").ap()
    hout_d = nc.dram_tensor("hout", [T, 64, 4, 128], F32, kind="ExternalOutput").ap()
    Cout_d = nc.dram_tensor("Cout", [64, 4, 128], F32, kind="ExternalOutput").ap()
    hTout_d = nc.dram_tensor("hTout", [128, 4, 64], F32, kind="ExternalOutput").ap()
    with tile.TileContext(nc) as tc, ExitStack() as ctx:
        singles = ctx.enter_context(tc.tile_pool(name="ls", bufs=1))
        preb = ctx.enter_context(tc.tile_pool(name="lp", bufs=2))
        houtb = ctx.enter_context(tc.tile_pool(name="lh", bufs=2))
        ps = ctx.enter_context(tc.tile_pool(name="lps", bufs=1, space="PSUM"))
        ps1 = ctx.enter_context(tc.tile_pool(name="lpt", bufs=2, space="PSUM"))
        whT = singles.tile([128, 4, G], F32R)
        nc.sync.dma_start(out=whT, in_=whT_d)
        idab = singles.tile([64, 64], BF16)
        nc.sync.dma_start(out=idab, in_=idab_d)
        idf = singles.tile([64, 64], F32)
        nc.sync.dma_start(out=idf, in_=idf_d)
        hT = singles.tile([128, 4, 64], F32R)
        nc.sync.dma_start(out=hT, in_=hT0_d)
        C = singles.tile([64, 4, 128], F32)
        nc.sync.dma_start(out=C, in_=C0_d)
        t_if = singles.tile([64, 4, 256], F32)
        t_g = singles.tile([64, 4, 128], F32)
        t_o = singles.tile([64, 4, 128], F32)
        stt1 = singles.tile([64, 4, 128], F32)
        tct = singles.tile([64, 4, 128], F32)
        for w in range(T // win):
            pre_sb = preb.tile([64, win, 4, 512], BF16, tag="pre")
            nc.sync.dma_start(
                out=pre_sb,
                in_=pre_d[w * win : (w + 1) * win].rearrange("t b j c -> b t j c"),
            )
            hout = houtb.tile([64, win, 4, 128], F32, tag="hout")
            for s in range(win):
                psum = ps.tile([64, 4, 512], F32, tag="gps")
                trps = ps1.tile([128, 4, 64], F32, tag="tps")
                for j in range(4):
                    nc.tensor.matmul(psum[:, j, :], idab, pre_sb[:, s, j, :],
                                     start=True, stop=False, skip_group_check=True)
                for j in range(4):
                    for k in range(4):
                        nc.tensor.matmul(psum[:, j, :], hT[:, k, :],
                                         whT[:, k, 512 * j : 512 * j + 512],
                                         start=False, stop=(k == 3),
                                         skip_group_check=True)
                nc.scalar.activation(t_if, psum[:, :, 0:256], TANH, scale=0.5)
                nc.scalar.activation(t_g, psum[:, :, 256:384], TANH, scale=1.0)
                nc.scalar.activation(t_o, psum[:, :, 384:512], TANH, scale=0.5)
                nc.vector.scalar_tensor_tensor(stt1, t_if[:, :, 0:128], 1.0, C, ADD, MULT)
                nc.vector.scalar_tensor_tensor(C, t_if[:, :, 128:256], 1.0, t_g, ADD, MULT)
                nc.vector.scalar_tensor_tensor(C, stt1, 0.5, C, MULT, ADD)
                nc.scalar.activation(tct, C, TANH, scale=0.5)
                nc.vector.scalar_tensor_tensor(hout[:, s, :, :], t_o, 1.0, tct, ADD, MULT)
                for j in range(4):
                    nc.tensor.transpose(trps[:, j, :], hout[:, s, j, :], idf)
                    nc.vector.tensor_copy(hT[:, j, :], trps[:, j, :])
            nc.sync.dma_start(
                out=hout_d[w * win : (w + 1) * win].rearrange("t b j c -> b t j c"),
                in_=hout,
            )
        nc.sync.dma_start(out=Cout_d, in_=C)
        nc.sync.dma_start(out=hTout_d, in_=hT[:, :, :].bitcast(F32))
    nc.compile()
    return nc


# ----------------------------------------------------------------- host glue

def perm_gates():
    idx = []
    for j in range(4):
        for gt in range(4):
            base = gt * H + 128 * j
            idx.extend(range(base, base + 128))
    return np.array(idx)


_PERM = perm_gates()


def pack_whT(W_hh):
    whT = 0.5 * W_hh[_PERM, :].T  # [H, G]
    return np.ascontiguousarray(whT.reshape(4, 128, G).transpose(1, 0, 2)).astype(np.float32)


def pack_pre(pre):
    return np.ascontiguousarray(pre[:, :, _PERM].reshape(SEQ, BATCH, 4, 512))


def run_gemm(x_cat, W_pair):
    """x_cat [SEQ, BATCH, 1024] fp32-ish; W_pair [2, G, 1024].
    Returns pre [2, SEQ, BATCH, G] fp32 (no bias)."""
    gemm = _cache.setdefault("gemm", build_gemm())
    xb = x_cat.astype(ml_dtypes.bfloat16)
    xT = np.ascontiguousarray(
        xb.reshape(SEQ * BATCH, IN).T
    )  # [1024, SEQ*BATCH]
    in_maps = []
    for core in range(8):
        d = core // 4       # direction
        q = core % 4        # time quarter
        sl = xT[:, q * TOK : (q + 1) * TOK]
        wT = np.ascontiguousarray(
            W_pair[d].T.reshape(8, 128, G).astype(ml_dtypes.bfloat16)
        ).transpose(1, 0, 2)  # [128, 8, G]
        in_maps.append({
            "xT": np.ascontiguousarray(sl),
            "wT": np.ascontiguousarray(wT),
        })
    res = run_bass_kernel_spmd(gemm, in_maps, core_ids=list(range(8)))
    pre = np.empty((2, SEQ, BATCH, G), np.float32)
    for core in range(8):
        d, q = core // 4, core % 4
        pre[d, q * TQ : (q + 1) * TQ] = (
            np.asarray(res.results[core]["out"], np.float32).reshape(TQ, BATCH, G)
        )
    return pre


def run_layer(pre_f, pre_b, Wh_f, Wh_b):
    """pre_* [SEQ, BATCH, G] fp32 WITH bias included. Returns h [SEQ, BATCH, 2H] fp32."""
    rec = _cache.setdefault("rec", build_rec())
    idab = np.eye(64, dtype=ml_dtypes.bfloat16)
    idf = np.eye(64, dtype=np.float32)
    hT0 = np.zeros((128, 4, 64), np.float32)
    maps = []
    for pre, Wh, rev in ((pre_f, Wh_f, False), (pre_b, Wh_b, True)):
        p = pre[::-1] if rev else pre
        maps.append({
            "pre_full": pack_pre(p).astype(ml_dtypes.bfloat16),
            "whT": pack_whT(Wh),
            "idab": idab,
            "idf": idf,
            "hT0": hT0,
            "C0": np.zeros((64, 4, 128), np.float32),
        })
    TC = 128
    houts = [[], []]
    for chunk in range(SEQ // TC):
        cmaps = []
        for ci in range(2):
            m = dict(maps[ci])
            m["pre"] = np.ascontiguousarray(m["pre_full"][chunk * TC : (chunk + 1) * TC])
            m.pop("pre_full")
            cmaps.append(m)
        res = run_bass_kernel_spmd(rec, cmaps, core_ids=[0, 1])
        for ci in range(2):
            houts[ci].append(np.asarray(res.results[ci]["hout"], np.float32))
            maps[ci]["C0"] = np.asarray(res.results[ci]["Cout"], np.float32)
            maps[ci]["hT0"] = np.asarray(res.results[ci]["hTout"], np.float32)
    hf = 0.5 * np.concatenate(houts[0]).reshape(SEQ, BATCH, H)
    hb = 0.5 * np.concatenate(houts[1]).reshape(SEQ, BATCH, H)
    hb = hb[::-1]
    return np.concatenate([hf, hb], axis=-1)


def kernel(x, W_ih, b_ih, b_hh, W_hh):
    x = np.asarray(x, np.float32)
    W_ih = np.asarray(W_ih, np.float32)
    W_hh = np.asarray(W_hh, np.float32)
    bias = np.asarray(b_ih, np.float32) + np.asarray(b_hh, np.float32)  # [2,2,G]

    # ---- layer 0 ----
    pre0 = run_gemm(x, W_ih[0])                      # [2, T, B, G]
    h0 = run_layer(pre0[0] + bias[0, 0], pre0[1] + bias[0, 1],
                   W_hh[0, 0], W_hh[0, 1])           # [T, B, 2H]
    # ---- layer 1 ----
    pre1 = run_gemm(h0, W_ih[1])
    h1 = run_layer(pre1[0] + bias[1, 0], pre1[1] + bias[1, 1],
                   W_hh[1, 0], W_hh[1, 1])
    return h1.astype(np.float32)

